# revision 15
# baseline (speedup 1.0000x reference)
"""Multi-head attention (B=16, N=577, C=768, H=12) on 8 TRN2 NeuronCores.

Strategy: pure data parallelism over batch (2 images per core, no
collectives). Per core, everything is computed "channels-on-partitions"
(transposed) so that no on-device transposes are ever needed:

  kT[outc, tok]   = wk8-pairs.T @ x8-pairs       (fp8e4m3 DoubleRow, K=256
                                                  per matmul -- K has no bias
                                                  and its quantization error
                                                  only jitters the logits)
  qT[outc, tok]   = wqT-tiles.T @ xT             (bf16; scaled 1/8 + bias on
                                                  evict)
  V[tok, outc]    = xT-tiles.T @ wvT             (bf16, natural layout, + bias)
  S^T[nk, nq]     = K^T-tiles.T @ Q^T            (bf16, K=64 contraction)
  E^T             = exp(S^T) * exp(relbT)        (host precomputes exp of the
                                                  transposed rel-pos bias)
  O'^T[65, nq]    = [V_h | 1]-tiles.T @ E^T      (row 64 = softmax denominator)
  O^T             = O'^T[0:64] * bcast(1/O'^T[64])
  out^T[co, tok]  = projT-tiles.T @ O^T + proj_b

Performance structure (baseline ~222 us):
  - K projection in fp8 DoubleRow (2 contraction planes per matmul, ~1.7x
    per-instruction speedup measured); Q / V / S / O' / out-proj stay bf16:
    fp8 anywhere on the V/output path or on BOTH of q,k pushes rel-err too
    close to the 2e-2 budget (measured 1.27e-2 with K-only)
  - DMA triggers cost ~620ns each serialized on the sync engine, so inputs
    are loaded with ONE dma_start per tensor (partition-major rearrange),
    rel-pos bias with two per head-PAIR, prefetched one pair ahead
  - all six fp8 K groups run first (only x8+wk8 needed, ~1MB), so the PE
    starts ~3us in while the bf16 tensors stream
  - heads processed in pairs (rows 0:64 / 64:128) so consecutive LDWEIGHTS
    alternate PE row groups and can overlap in-flight matmuls
  - the five 65-wide S rump chunks share one PSUM bank and are evicted by a
    single strided exp; exp IS the PSUM evict (fused on ScalarE); the
    rel-pos multiply is one wide in-place bf16 op on VectorE; K evicts run
    during startup while ScalarE is otherwise idle
  - O' is evicted to SBUF immediately (frees PSUM banks) and the whole
    normalize chain runs out of SBUF off the critical path
  - batch-0 output projection spread across early batch-1 pairs; batch-1
    output projection split into a ki=0..4 partial (P1, interleaved into
    the last attention pair as PE filler, evicted to a f32 accumulator)
    plus a ki=5 finish (P2) with pipelined DVE combines -- the tail no
    longer serializes the full projection behind the last softmax
  - custom-DVE ops (reciprocal_approx_fast) and partition_broadcast read
    physical partition 0 regardless of the AP base -> denominators are
    staged to a base-0 row first

Host side pre-transposes all inputs (bf16 everywhere except the fp8
contraction-pair-packed K weights/activations) and transposes the output
back. PSUM accumulation is f32 throughout.
"""
import numpy as np
import ml_dtypes

B, N, C, H, HD = 16, 577, 768, 12, 64
NCORES = 8
BPC = B // NCORES          # batches per core: 2
NT = BPC * N               # tokens per core: 1154
P = 128

# token-free-dim chunks over NT (matmul free dim <= 512 for f32 psum)
TFREE = [(0, 512), (512, 512), (1024, 130)]
# nk (key token) tiles over N
NKT = [(0, 128), (128, 128), (256, 128), (384, 128), (512, 65)]

_CACHE = {}


def _build():
    import concourse.tile as tile
    from concourse import bacc, mybir

    bf16 = mybir.dt.bfloat16
    f8 = mybir.dt.float8e4
    f32 = mybir.dt.float32
    Alu = mybir.AluOpType
    Act = mybir.ActivationFunctionType
    DR = mybir.MatmulPerfMode.DoubleRow

    nc = bacc.Bacc(
        "TRN2",
        target_bir_lowering=False,
        debug=False,
        enable_asserts=False,
        num_devices=NCORES,
    )
    x8 = nc.dram_tensor("x8", [3 * P, 2 * NT], f8, kind="ExternalInput").ap()
    wk8 = nc.dram_tensor("wk8", [3 * P, 2 * C], f8, kind="ExternalInput").ap()
    xT = nc.dram_tensor("xT", [C, NT], bf16, kind="ExternalInput").ap()
    wqT = nc.dram_tensor("wqT", [C, C], bf16, kind="ExternalInput").ap()
    wvT = nc.dram_tensor("wvT", [C, C], bf16, kind="ExternalInput").ap()
    qbias = nc.dram_tensor("qbias", [P, 6], f32, kind="ExternalInput").ap()
    vbias = nc.dram_tensor("vbias", [1, C], f32, kind="ExternalInput").ap()
    relbT = nc.dram_tensor("relbT", [H, N, N], bf16, kind="ExternalInput").ap()
    projT = nc.dram_tensor("projT", [C, C], bf16, kind="ExternalInput").ap()
    pbias = nc.dram_tensor("pbias", [P, 6], f32, kind="ExternalInput").ap()
    out = nc.dram_tensor("out", [C, NT], f32, kind="ExternalOutput").ap()

    with tile.TileContext(nc) as tc:
        with (
            tc.tile_pool(name="persist", bufs=1) as pp,
            tc.tile_pool(name="relb", bufs=2) as relp,
            tc.tile_pool(name="st", bufs=2) as stp,
            tc.tile_pool(name="dn", bufs=2) as dnp,
            tc.tile_pool(name="oev", bufs=4) as oevp,
            tc.tile_pool(name="oevr", bufs=2) as oevrp,
            tc.tile_pool(name="psbig", bufs=4, space="PSUM") as ps_big,
            tc.tile_pool(name="psrump", bufs=1, space="PSUM") as ps_r,
            tc.tile_pool(name="pso", bufs=2, space="PSUM") as ps_o,
        ):
            # ---------------- Phase A: load weights / constants ----------
            # ONE dma_start per tensor (triggers are ~620ns serialized on
            # the sync engine). fp8 K inputs first: the PE's first work.
            x8a = pp.tile([P, 3 * 2 * NT], f8, tag="x8", name="x8")
            nc.sync.dma_start(
                x8a[:, : 2 * 2 * NT].rearrange("p (t f) -> p t f", t=2),
                x8[: 2 * P, :].rearrange("(t p) f -> p t f", p=P),
            )
            nc.sync.dma_start(x8a[:, 2 * 2 * NT :], x8[2 * P :, :])
            x8t = [
                x8a[:, :].rearrange("p (t two f) -> p t two f", t=3, two=2)[:, i]
                for i in range(3)
            ]
            w8a = pp.tile([P, 3 * 2 * C], f8, tag="w8", name="w8")
            nc.sync.dma_start(
                w8a[:, :].rearrange("p (t f) -> p t f", t=3),
                wk8[:, :].rearrange("(t p) f -> p t f", p=P),
            )
            w8t = [
                w8a[:, :].rearrange("p (t two m) -> p t two m", t=3, two=2)[:, i]
                for i in range(3)
            ]
            qb = pp.tile([P, 6], f32, tag="qb", name="qb")
            nc.sync.dma_start(qb[:], qbias[:])
            pb = pp.tile([P, 6], f32, tag="pb", name="pb")
            nc.sync.dma_start(pb[:], pbias[:])
            vbr = pp.tile([1, C], f32, tag="vbr", name="vbr")
            nc.sync.dma_start(vbr[:], vbias[:])
            vb = pp.tile([P, C], f32, tag="vb", name="vb")
            nc.gpsimd.partition_broadcast(vb[:, :], vbr[0:1, :])

            xta = pp.tile([P, 6 * NT], bf16, tag="xt", name="xt")
            nc.sync.dma_start(
                xta[:, :].rearrange("p (t f) -> p t f", t=6),
                xT[:, :].rearrange("(t p) f -> p t f", p=P),
            )
            xt = [xta[:, NT * i : NT * (i + 1)] for i in range(6)]
            wqa = pp.tile([P, 6 * C], bf16, tag="wq", name="wq")
            nc.sync.dma_start(
                wqa[:, :].rearrange("p (t f) -> p t f", t=6),
                wqT[:, :].rearrange("(t p) f -> p t f", p=P),
            )
            wqt = [wqa[:, C * i : C * (i + 1)] for i in range(6)]
            wva = pp.tile([P, 6 * C], bf16, tag="wv", name="wv")
            nc.sync.dma_start(
                wva[:, :].rearrange("p (t f) -> p t f", t=6),
                wvT[:, :].rearrange("(t p) f -> p t f", p=P),
            )
            wvt = [wva[:, C * i : C * (i + 1)] for i in range(6)]
            pta = pp.tile([P, 6 * C], bf16, tag="pt", name="pt")
            pt = [pta[:, C * i : C * (i + 1)] for i in range(6)]

            # ---------------- persistent result tiles ---------------------
            # qk[t] for t in 0..11: [128, NT] bf16, outc block t (q: 0-5, k: 6-11)
            qk = []
            for t in range(12):
                qk.append(pp.tile([P, NT], bf16, tag=f"qk{t}", name=f"qk{t}"))
            # o[t]: [128, NT] bf16 -- O^T assembled for the projection
            o = []
            for t in range(6):
                o.append(pp.tile([P, NT], bf16, tag=f"o{t}", name=f"o{t}"))
            v = [[None] * 5 for _ in range(BPC)]

            def k_group(t):
                # K^T projection block t (outc 128t..): fp8 DoubleRow,
                # 3 matmuls of K=256; plain-copy evict (K has no bias)
                for (f0, fsz) in TFREE:
                    ps = ps_big.tile([P, 512], f32, tag="big", name="psmm")
                    for p8 in range(3):
                        nc.tensor.matmul(
                            ps[:, 0:fsz],
                            w8t[p8][:, :, P * t : P * (t + 1)],
                            x8t[p8][:, :, f0 : f0 + fsz],
                            start=(p8 == 0),
                            stop=(p8 == 2),
                            perf_mode=DR,
                        )
                    nc.scalar.copy(qk[6 + t][:, f0 : f0 + fsz], ps[:, 0:fsz])

            def q_group(t):
                # Q^T projection block t: bf16, 6 K=128 matmuls; evict on
                # ACT with 1/8 scale + (pre-scaled) bias
                for (f0, fsz) in TFREE:
                    ps = ps_big.tile([P, 512], f32, tag="big", name="psmm")
                    for ki in range(6):
                        nc.tensor.matmul(
                            ps[:, 0:fsz],
                            wqt[ki][:, P * t : P * (t + 1)],
                            xt[ki][:, f0 : f0 + fsz],
                            start=(ki == 0),
                            stop=(ki == 5),
                        )
                    nc.scalar.activation(
                        qk[t][:, f0 : f0 + fsz],
                        ps[:, 0:fsz],
                        Act.Identity,
                        bias=qb[:, t : t + 1],
                        scale=0.125,
                    )

            def v_group(b, j):
                # V projection (natural layout) for batch b, token tile j
                # v[b][j]: [nksz, 780] bf16, 12 head-blocks of 65 (64 V + ones)
                nk0, nksz = NKT[j]
                vt = pp.tile([P, 12 * 65], bf16, tag=f"v{b}_{j}", name=f"v{b}_{j}")
                v[b][j] = vt
                v3 = vt[:, :].rearrange("p (h w) -> p h w", w=65)
                nc.gpsimd.memset(v3[:, :, 64:65], 1.0)
                tok0 = b * N + nk0
                for half in range(2):  # outc halves of 384 = 6 heads
                    f0 = 384 * half
                    ps = ps_big.tile([P, 512], f32, tag="big", name="psmm")
                    for ki in range(6):
                        nc.tensor.matmul(
                            ps[0:nksz, 0:384],
                            xt[ki][:, tok0 : tok0 + nksz],
                            wvt[ki][:, f0 : f0 + 384],
                            start=(ki == 0),
                            stop=(ki == 5),
                        )
                    ps3 = ps[0:nksz, 0:384].rearrange("p (h w) -> p h w", w=64)
                    vb3 = vb[0:nksz, f0 : f0 + 384].rearrange(
                        "p (h w) -> p h w", w=64
                    )
                    nc.vector.tensor_tensor(
                        v3[0:nksz, 6 * half : 6 * half + 6, 0:64],
                        ps3[:, :, :],
                        vb3[:, :, :],
                        op=Alu.add,
                    )

            _projst = {}

            def proj_chunk(t, f0, fsz):
                # one psum chunk of the batch-0 output projection; the two
                # chunks of a t share one oev tile -> ONE out DMA at the end
                if t not in _projst:
                    _projst[t] = oevp.tile([P, N], f32, tag="oev", name="oev")
                ot = _projst[t]
                ps = ps_big.tile([P, 512], f32, tag="big", name="psmm")
                for ki in range(6):
                    nc.tensor.matmul(
                        ps[:, 0:fsz],
                        pt[ki][:, P * t : P * (t + 1)],
                        o[ki][:, f0 : f0 + fsz],
                        start=(ki == 0),
                        stop=(ki == 5),
                    )
                nc.scalar.activation(
                    ot[:, f0 : f0 + fsz],
                    ps[:, 0:fsz],
                    Act.Identity,
                    bias=pb[:, t : t + 1],
                )
                if f0 + fsz == N:
                    nc.sync.dma_start(out[P * t : P * (t + 1), 0:N], ot[:, 0:N])
                    del _projst[t]

            def proj_group(t):
                proj_chunk(t, 0, 512)
                proj_chunk(t, 512, 65)

            # batch-1 projection: ki 0..4 accumulate into psum banks that
            # stay OPEN (P1); ki 5 finishes in-place once o[5] lands (P2);
            # one biased ACT evict per t, no recombine pass. Waves of 3 t
            # (3 big banks + one shared 65-rump bank) fit alongside the
            # last pair's O' psum.
            _p1ps = {}
            _p1rump = {}

            def p1_wave(t, wave):
                # 512-chunk only: ONE open accumulation group per psum bank
                # (the 65-rumps are done with closed groups in p2_wave --
                # hardware allows a single open group per bank)
                ps = ps_big.tile([P, 512], f32, tag="big", name="psmm")
                _p1ps[t] = ps
                for ki in range(5):
                    nc.tensor.matmul(
                        ps[:, 0:512],
                        pt[ki][:, P * t : P * (t + 1)],
                        o[ki][:, N : N + 512],
                        start=(ki == 0),
                        stop=False,
                    )

            def p2_wave(t, wave):
                ps = _p1ps.pop(t)
                if wave not in _p1rump:
                    _p1rump[wave] = ps_r.tile(
                        [P, 512], f32, tag=f"rump{wave % 2}", name=f"p1r{wave}"
                    )
                pr = _p1rump[wave]
                i = t % 3
                nc.tensor.matmul(
                    ps[:, 0:512],
                    pt[5][:, P * t : P * (t + 1)],
                    o[5][:, N : N + 512],
                    start=False,
                    stop=True,
                )
                for ki in range(6):
                    nc.tensor.matmul(
                        pr[:, 65 * i : 65 * i + 65],
                        pt[ki][:, P * t : P * (t + 1)],
                        o[ki][:, N + 512 : N + N],
                        start=(ki == 0),
                        stop=(ki == 5),
                    )
                ot = oevp.tile([P, N], f32, tag="oev", name="oev")
                nc.scalar.activation(
                    ot[:, 0:512], ps[:, 0:512], Act.Identity, bias=pb[:, t : t + 1]
                )
                nc.scalar.activation(
                    ot[:, 512:577],
                    pr[:, 65 * i : 65 * i + 65],
                    Act.Identity,
                    bias=pb[:, t : t + 1],
                )
                nc.sync.dma_start(out[P * t : P * (t + 1), N : N + N], ot[:, 0:N])

            # ---- rel-pos bias: one [128, 2*5*N] tile per pair, 2 DMAs ----
            def rb_fetch(h0, eng=None):
                # DMA APs are limited to 3 dims, so load per head (2+2
                # triggers per pair) into the shared pair tile
                eng = eng or nc.sync
                t = relp.tile([P, 2 * 5 * N], bf16, tag="rbp", name=f"rbp{h0}")
                for hh in (h0, h0 + 1):
                    hs = t[:, (hh - h0) * 5 * N : (hh - h0 + 1) * 5 * N]
                    h3 = hs.rearrange("p (j q) -> p j q", q=N)
                    eng.dma_start(
                        h3[:, 0:4, :],
                        relbT[hh, 0:512, :].rearrange("(j p) q -> p j q", p=P),
                    )
                    eng.dma_start(h3[0:65, 4, :], relbT[hh, 512:577, :])
                return t

            def attention_pair(b, h0, rbp, fillers=None):
                # heads h0 (rows 0:64) and h0+1 (rows 64:128) interleaved so
                # consecutive LDWEIGHTS alternate PE row groups (overlap).
                fillers = fillers if fillers is not None else []

                def drain():
                    if fillers:
                        fillers.pop(0)()

                qt = h0 // 2
                rba = {
                    hh: rbp[:, :].rearrange("p (h x) -> p h x", h=2)[:, hh - h0]
                    for hh in (h0, h0 + 1)
                }
                sta = {}
                rump = {}
                for hh in (h0, h0 + 1):
                    pr = hh % 2
                    sta[hh] = stp.tile([P, 5 * N], bf16, tag=f"sta{pr}", name=f"sta{pr}")
                    rump[hh] = ps_r.tile([P, 512], f32, tag=f"rump{pr}", name=f"rump{pr}")
                for j, (nk0, nksz) in enumerate(NKT):
                    for hh in (h0, h0 + 1):
                        qoff = (hh % 2) * 64
                        lk = qk[6 + qt][qoff : qoff + 64, b * N + nk0 : b * N + nk0 + nksz]
                        ps = ps_big.tile([P, 512], f32, tag="big", name="pss")
                        nc.tensor.matmul(
                            ps[0:nksz, 0:512],
                            lk,
                            qk[qt][qoff : qoff + 64, b * N : b * N + 512],
                            start=True,
                            stop=True,
                        )
                        nc.tensor.matmul(
                            rump[hh][0:nksz, 65 * j : 65 * j + 65],
                            lk,
                            qk[qt][qoff : qoff + 64, b * N + 512 : b * N + N],
                            start=True,
                            stop=True,
                        )
                        # exp-evict of the 512-wide chunk on ACT
                        nc.scalar.activation(
                            sta[hh][0:nksz, N * j : N * j + 512],
                            ps[0:nksz, 0:512],
                            Act.Exp,
                        )
                    drain()
                for hh in (h0, h0 + 1):
                    # strided exp-evict for the 65-wide rumps, split to match
                    # the bias-mult split (blocks 0-2, then 3-4)
                    rump3 = rump[hh][:, 0:325].rearrange("p (j q) -> p j q", q=65)
                    sta3 = sta[hh][:, :].rearrange("p (j q) -> p j q", q=N)
                    nc.scalar.activation(
                        sta3[:, 0:3, 512:577], rump3[:, 0:3, :], Act.Exp
                    )
                    nc.scalar.activation(
                        sta3[:, 3:5, 512:577], rump3[:, 3:5, :], Act.Exp
                    )
                    # multiplicative bias on DVE, split so the O' matmuls of
                    # the first blocks can start before the last exp lands
                    nc.vector.tensor_tensor(
                        sta[hh][:, 0 : 3 * N],
                        sta[hh][:, 0 : 3 * N],
                        rba[hh][:, 0 : 3 * N],
                        op=Alu.mult,
                    )
                    nc.vector.tensor_tensor(
                        sta[hh][:, 3 * N : 5 * N],
                        sta[hh][:, 3 * N : 5 * N],
                        rba[hh][:, 3 * N : 5 * N],
                        op=Alu.mult,
                    )
                for hh in (h0, h0 + 1):
                    qoff = (hh % 2) * 64
                    # O'^T = [V_hh | 1]-tiles.T @ E^T in two 289/288 chunks
                    ost = dnp.tile([65, N], f32, tag="ost", name="ost")
                    psoA = ps_o.tile([65, 289], f32, tag="o", name="psoA")
                    psoB = ps_o.tile([65, 289], f32, tag="o", name="psoB")
                    for j, (nk0, nksz) in enumerate(NKT):
                        lv = v[b][j][0:nksz, 65 * hh : 65 * hh + 65]
                        nc.tensor.matmul(
                            psoA[0:65, 0:289],
                            lv,
                            sta[hh][0:nksz, N * j : N * j + 289],
                            start=(j == 0),
                            stop=(j == 4),
                        )
                        nc.tensor.matmul(
                            psoB[0:65, 0:288],
                            lv,
                            sta[hh][0:nksz, N * j + 289 : N * j + N],
                            start=(j == 0),
                            stop=(j == 4),
                        )
                    drain()
                    # early evict to SBUF (frees the psum banks fast); the
                    # whole normalize chain then runs out of SBUF
                    nc.vector.tensor_copy(ost[0:65, 0:289], psoA[0:65, 0:289])
                    nc.scalar.copy(ost[0:65, 289:577], psoB[0:65, 0:288])
                    dr = dnp.tile([1, N], f32, tag="dr", name="dr")
                    nc.vector.tensor_copy(dr[0:1, 0:N], ost[64:65, 0:N])
                    rr = dnp.tile([1, N], f32, tag="rr", name="rr")
                    nc.vector.reciprocal_approx_fast(rr[0:1, 0:N], dr[0:1, 0:N])
                    rb = dnp.tile([64, N], f32, tag="rbb", name="rbb")
                    nc.gpsimd.partition_broadcast(rb[0:64, 0:N], rr[0:1, 0:N])
                    nc.vector.tensor_tensor(
                        o[qt][qoff : qoff + 64, b * N : b * N + N],
                        ost[0:64, 0:N],
                        rb[0:64, 0:N],
                        op=Alu.mult,
                    )
                while fillers:
                    fillers.pop(0)()

            # ------------- interleaved emission schedule -------------------
            # all fp8 K groups first (only ~1MB of input needed), then Q0
            # and V(b0); per pair: Q prefetch right before its heads, V(b1)
            # spread across late b0 heads, proj(b0) spread across early b1
            # pairs, P1 partials inside the last pair, P2 at the end.
            # rel-pos bias is prefetched one pair ahead.
            for t in range(6):
                k_group(t)
            q_group(0)
            rbp_cur = rb_fetch(0)
            for j in range(5):
                v_group(0, j)
            for b in range(BPC):
                for h0 in range(0, 12, 2):
                    if b == 0 and h0 < 10:       # prefetch next pair's Q
                        q_group(h0 // 2 + 1)
                    # prefetch next pair's rel-pos bias
                    nh = h0 + 2 if h0 < 10 else (0 if b == 0 else None)
                    rbp_next = rb_fetch(nh) if nh is not None else None
                    fillers = []
                    if b == 1 and h0 == 0:
                        # last two V(b1) tiles fill pair(1,0)'s S phase
                        # (their O' consumers come after in the PE queue)
                        fillers.append(lambda: v_group(1, 3))
                        fillers.append(lambda: v_group(1, 4))
                    if b == 1 and h0 <= 8:
                        # spread batch-0 out-proj chunks as fillers: two
                        # chunk-groups per pair, t=0..4 (t=5 goes last)
                        cs = [(0, 0, 512), (0, 512, 65), (1, 0, 512),
                              (1, 512, 65), (2, 0, 512), (2, 512, 65),
                              (3, 0, 512), (3, 512, 65), (4, 0, 512),
                              (4, 512, 65)][h0 : h0 + 2]
                        for (t, f0, fsz) in cs:
                            fillers.append(
                                (lambda t=t, f0=f0, fsz=fsz: proj_chunk(t, f0, fsz))
                            )
                    if b == 1 and h0 == 10:
                        # last b0-proj chunks fill the final pair's S phase
                        fillers.append(lambda: proj_chunk(5, 0, 512))
                        fillers.append(lambda: proj_chunk(5, 512, 65))
                    attention_pair(b, h0, rbp_cur, fillers)
                    rbp_cur = rbp_next
                    if b == 0 and h0 == 4:
                        # proj weights: not needed until the batch-1 phase
                        nc.sync.dma_start(
                            pta[:, :].rearrange("p (t f) -> p t f", t=6),
                            projT[:, :].rearrange("(t p) f -> p t f", p=P),
                        )
                    if b == 0 and h0 >= 6:
                        v_group(1, (h0 - 6) // 2)          # V(b1) late in b0
            # ---------------- batch-1 projection finish --------------------
            # wave A P1 runs on the PE while the last pair's softmax
            # normalize chain drains on DVE/GpSimd; P2 then needs o[5]
            for w, (ta, tb) in enumerate(((0, 1), (2, 3), (4, 5))):
                p1_wave(ta, w)
                p1_wave(tb, w)
                p2_wave(ta, w)
                p2_wave(tb, w)

    nc.compile()
    return nc


def _get_nc():
    if "nc" not in _CACHE:
        _CACHE["nc"] = _build()
    return _CACHE["nc"]


def make_in_maps(x, rel_pos_bias, qkv_w, q_bias, v_bias, proj_w, proj_b):
    bf = ml_dtypes.bfloat16
    f8 = ml_dtypes.float8_e4m3
    x = np.asarray(x, dtype=np.float32)
    rel_pos_bias = np.asarray(rel_pos_bias, dtype=np.float32)
    qkv_w = np.asarray(qkv_w, dtype=np.float32)
    q_bias = np.asarray(q_bias, dtype=np.float32)
    v_bias = np.asarray(v_bias, dtype=np.float32)
    proj_w = np.asarray(proj_w, dtype=np.float32)
    proj_b = np.asarray(proj_b, dtype=np.float32)

    # fp8 K weights, contraction-pair packed: [3*128, 2*768]
    # pair p plane i = input chans 256p+128i..+128
    wkT = np.ascontiguousarray(qkv_w[C : 2 * C].T)                      # [768, 768]
    wk8 = np.zeros((3 * P, 2 * C), dtype=f8)
    for p in range(3):
        wk8[P * p : P * (p + 1), 0:C] = wkT[256 * p : 256 * p + P].astype(f8)
        wk8[P * p : P * (p + 1), C:] = wkT[256 * p + P : 256 * p + 256].astype(f8)

    wqT = np.ascontiguousarray(qkv_w[:C].T).astype(bf)                  # [768, 768]
    wvT = np.ascontiguousarray(qkv_w[2 * C :].T).astype(bf)             # [768, 768]
    qbias = np.ascontiguousarray((q_bias * 0.125).reshape(6, P).T)      # [128, 6]
    vbias = np.ascontiguousarray(v_bias[None, :])                       # [1, 768]
    # exp of the transposed rel-pos bias: applied multiplicatively after exp(S)
    relbT = np.ascontiguousarray(
        np.exp(rel_pos_bias[0].transpose(0, 2, 1))
    ).astype(bf)
    projT = np.ascontiguousarray(proj_w.T).astype(bf)                   # [768, 768]
    pbias = np.ascontiguousarray(proj_b.reshape(6, P).T)                # [128, 6]

    in_maps = []
    for c in range(NCORES):
        xTc = np.ascontiguousarray(
            x[BPC * c : BPC * (c + 1)].reshape(NT, C).T
        )                                                               # [768, 1154]
        x8 = np.zeros((3 * P, 2 * NT), dtype=f8)
        for p in range(3):
            x8[P * p : P * (p + 1), 0:NT] = xTc[256 * p : 256 * p + P].astype(f8)
            x8[P * p : P * (p + 1), NT:] = xTc[256 * p + P : 256 * p + 256].astype(f8)
        in_maps.append(
            dict(
                x8=x8,
                wk8=wk8,
                xT=xTc.astype(bf),
                wqT=wqT,
                wvT=wvT,
                qbias=qbias,
                vbias=vbias,
                relbT=relbT,
                projT=projT,
                pbias=pbias,
            )
        )
    return in_maps


def kernel(x, rel_pos_bias, qkv_w, q_bias, v_bias, proj_w, proj_b):
    from concourse import bass_utils

    in_maps = make_in_maps(x, rel_pos_bias, qkv_w, q_bias, v_bias, proj_w, proj_b)
    nc = _get_nc()
    res = bass_utils.run_bass_kernel_spmd(nc, in_maps, core_ids=list(range(NCORES)))
    outs = []
    for c in range(NCORES):
        oT = res.results[c]["out"]                                      # [768, 1154]
        outs.append(np.ascontiguousarray(oT.T).reshape(BPC, N, C))
    return np.concatenate(outs, axis=0)
